# Initial kernel scaffold
#
"""Trainium2 Bass kernel for nn_AXSLinearMixedPrecision (auto-assembled).

out = fake_quant(bf16(x)) @ fake_quant(bf16(W)).T + bf16(bias), blockwise
(block=32) absmax fake-quant on the [-15, 15] grid, bf16 GEMM + output.

Distribution over 8 NeuronCores: tensor-parallel on W rows (out_features
sharded 8 x 2048), x replicated; each core produces a 2048-wide output
column slice; the host concatenates.
"""



import orjson

from concourse import tile
from concourse.vector_clock import ScopedClock

MAX_WAITS = 1
DRAIN_MAX_WAITS = 1


def split_bir_waits(bir: dict, max_waits: int = 1) -> int:
    """Enforce <= max_waits semaphore waits per BIR instruction.

    Excess waits move onto NoOp carriers inserted immediately before the
    instruction on the same engine.  Tile emits per-proc streams in global
    tick order, so every wait references strictly-earlier work — blocking
    the dispatching engine a bit earlier cannot deadlock.
    """
    n_split = 0
    n_carrier = 0
    for f in bir.get("functions", []):
        for bb in f.get("blocks", []):
            insts = bb.get("instructions", [])
            out = []
            for ins in insts:
                si = ins.get("sync_info")
                waits = (si or {}).get("on_wait") or []
                if len(waits) > max_waits:
                    n_split += 1
                    keep = waits[len(waits) - max_waits :]
                    moved = waits[: len(waits) - max_waits]
                    for i in range(0, len(moved), max_waits):
                        n_carrier += 1
                        out.append(
                            {
                                "engine": ins.get("engine", "SP"),
                                "ins": [],
                                "outs": [],
                                "name": f"WSPLIT-{n_carrier}",
                                "opcode": "NoOp",
                                "sync_info": {
                                    "on_update": [],
                                    "on_wait": moved[i : i + max_waits],
                                },
                                "text_hint": "wait_split",
                            }
                        )
                    si["on_wait"] = keep
                out.append(ins)
            bb["instructions"] = out
    return n_split


_patched = False


def install_wait_split_hook():
    """Rewrite the BIR between bass serialization and walrus codegen."""
    global _patched
    if _patched:
        return
    _patched = True
    import concourse.bass2jax as b2j

    orig = b2j.compile_bir_kernel

    def compile_with_split(ant_bir, *args, **kwargs):
        bir = orjson.loads(ant_bir)
        split_bir_waits(bir, MAX_WAITS)
        return orig(orjson.dumps(bir), *args, **kwargs)

    b2j.compile_bir_kernel = compile_with_split


class CompatTileContext(tile.TileContext):
    def _drain_and_barrier(self, tick_clock, wait_clock):
        nc = self.nc
        drain_inst = nc.sync.drain()
        wait_clock.add_sem_waits(
            drain_inst.ins, ScopedClock({None: tick_clock.global_clock})
        )
        waits = list(drain_inst.ins.sync_info.on_wait)
        if len(waits) > DRAIN_MAX_WAITS:
            drain_inst.ins.sync_info.on_wait = waits[:DRAIN_MAX_WAITS]
            rest = waits[DRAIN_MAX_WAITS:]
            import bass_rust

            for i in range(0, len(rest), MAX_WAITS):
                nop = nc.sync.nop(nofuse=True, hint="drain_wait_split")
                nop.ins.sync_info = bass_rust.SyncInfo(
                    on_wait=rest[i : i + MAX_WAITS], on_update=[]
                )

        nc.all_engine_barrier()
        assert self.sems is not None
        popped = nc._tile_sem_poison_stack.pop()
        assert popped is self._sem_poison
        nc.clear_and_free_semaphores(list(self.sems.allocated().values()))
        nc.all_engine_barrier()



import concourse.bass as bass
import concourse.mybir as mybir

F32 = mybir.dt.float32
BF16 = mybir.dt.bfloat16
BLK = 32
QMAX = 15.0
MAGIC = 12582912.0  # 1.5 * 2**23: RNE integer rounding for |v| < 2**22
K1 = 0.0666656494140625  # floor((1/15)*2^19)/2^19 — 16-bit mantissa
K2 = 1.0172370821237564e-06  # floor(((1/15)-K1)*2^35)/2^35 — 16-bit mantissa
AX = mybir.AxisListType.X
OP = mybir.AluOpType


def build_kernel(
    M, K, N, n_free=512, dram_bufs=4, k_chunk_subtiles=8, sbuf_transpose=True
):
    P = 128
    KS = K // P         # k-subtiles
    MT = M // P         # x row tiles
    NT = N // P         # w row tiles
    NF = N // n_free    # psum tiles / wqT slices
    NB = K // BLK       # quant blocks per row
    NPT = NT // NF      # w row tiles per wqT slice
    KCH = min(k_chunk_subtiles, KS)   # k-subtiles per transpose chunk
    NKC = KS // KCH                   # transpose chunks along K
    if NKC < 2:                       # one chunk per K-half minimum
        KCH = KS // 2
        NKC = 2
    assert K % P == 0 and M % P == 0 and N % P == 0 and N % n_free == 0
    assert NT % NF == 0 and KS % KCH == 0

    nc = bass.Bass(target_bir_lowering=False)
    x = nc.dram_tensor("x", [M, K], BF16, kind="ExternalInput")
    w = nc.dram_tensor("w", [N, K], BF16, kind="ExternalInput")
    b = nc.dram_tensor("bias", [1, N], BF16, kind="ExternalInput")
    out = nc.dram_tensor("out", [M, N], BF16, kind="ExternalOutput")

    with CompatTileContext(nc) as tc:
        with (
            tc.tile_pool(name="resident", bufs=1) as resident,
            tc.tile_pool(name="xio", bufs=2) as xio,
            tc.tile_pool(name="stats", bufs=1) as stats,
            tc.tile_pool(name="t32p", bufs=1) as t32p,
            tc.tile_pool(name="xqt", bufs=4) as xqtp,
            tc.tile_pool(name="outp", bufs=2) as outp,
            tc.tile_pool(name="psum", bufs=8, space="PSUM") as psump,
            tc.tile_pool(name="dram", bufs=dram_bufs, space="DRAM") as dramp,
        ):
            wqT = [
                resident.tile([P, KS, n_free], BF16, name=f"wqT_{p}")
                for p in range(NF)
            ]
            bias_sb = resident.tile([P, N], BF16)
            nc.sync.dma_start(bias_sb[:], b[:].to_broadcast((P, N)))
            magic_sb = resident.tile([P, 1], F32)
            nc.vector.memset(magic_sb[:], MAGIC)

            def quantize(t_in, emit_half, tagn):
                """Quantize [128, K] bf16 t_in; emit_half(h, xq_half) gets
                each quantized K-half ([128, K/2] bf16) so its transposes
                start while the other half is still quantizing."""
                v_in = t_in.rearrange("p (b i) -> p b i", i=BLK)
                a = stats.tile([P, NB], F32, tag="stat_a")
                nc.vector.tensor_reduce(
                    a[:], v_in, axis=AX, op=OP.max, apply_absolute_value=True
                )
                # s = RN(amax/15) exactly: amax has an 8-bit mantissa (|bf16|),
                # so amax*K1 and amax*K2 are exact and their sum rounds to the
                # true quotient (K1+K2 carry 32 bits of 1/15).
                nc.vector.tensor_scalar_max(a[:], a[:], 1e-30)
                u1 = stats.tile([P, NB], F32, tag="stat_ur", name=f"u1_{tagn}")
                nc.vector.tensor_scalar_mul(u1[:], a[:], K1)
                s = stats.tile([P, NB], F32, tag="stat_s")
                nc.vector.scalar_tensor_tensor(
                    s[:], a[:], K2, u1[:], op0=OP.mult, op1=OP.add
                )
                r = stats.tile([P, NB], F32, tag="stat_ur", name=f"r_{tagn}")
                nc.vector.reciprocal(r[:], s[:])
                KH = K // 2
                NBH = NB // 2
                for h in range(2):
                    vh = t_in[:, h * KH : (h + 1) * KH].rearrange(
                        "p (b i) -> p b i", i=BLK
                    )
                    rh = r[:, h * NBH : (h + 1) * NBH, None].to_broadcast(
                        (P, NBH, BLK)
                    )
                    sh = s[:, h * NBH : (h + 1) * NBH, None].to_broadcast(
                        (P, NBH, BLK)
                    )
                    t = t32p.tile([P, KH], F32, tag="t32", name=f"t32_{tagn}_{h}")
                    tv = t.rearrange("p (b i) -> p b i", i=BLK)
                    nc.vector.tensor_tensor(tv, vh, rh, op=OP.mult)
                    nc.scalar.activation(
                        t[:], t[:], mybir.ActivationFunctionType.Identity,
                        bias=magic_sb[:],
                    )
                    xq_h = xio.tile(
                        [P, KH], BF16, tag="xq_sb", name=f"xqh_{tagn}_{h}"
                    )
                    qv = xq_h.rearrange("p (b i) -> p b i", i=BLK)
                    nc.vector.scalar_tensor_tensor(
                        qv, tv, MAGIC, sh, op0=OP.subtract, op1=OP.mult
                    )
                    emit_half(h, xq_h)

            def x_quant_transpose(mt):
                x_in = xio.tile([P, K], BF16, tag="x_in", name=f"x_in_{mt}")
                nc.sync.dma_start(x_in[:], x[mt * P : (mt + 1) * P, :])
                xqT = [None] * NKC
                CPH = NKC // 2  # transpose chunks per K-half

                def emit_half(h, xq_h):
                    for cc in range(CPH):
                        c = h * CPH + cc
                        xt = xqtp.tile(
                            [P, KCH, P], BF16, tag=f"xqT{c}", name=f"xqT{c}_{mt}"
                        )
                        nc.sync.dma_start_transpose(
                            xt[:], xq_h[:, cc * KCH * P : (cc + 1) * KCH * P]
                        )
                        xqT[c] = xt

                quantize(x_in, emit_half, f"x{mt}")
                return xqT

            def quant_w_tile(nt):
                p, ntl = nt // NPT, nt % NPT
                w_in = xio.tile([P, K], BF16, tag="x_in", name=f"w_in_{nt}")
                nc.sync.dma_start(w_in[:], w[nt * P : (nt + 1) * P, :])
                CPH = NKC // 2

                def emit_half(h, wq_h):
                    for cc in range(CPH):
                        c = h * CPH + cc
                        nc.sync.dma_start_transpose(
                            wqT[p][
                                :, c * KCH : (c + 1) * KCH,
                                ntl * P : (ntl + 1) * P,
                            ],
                            wq_h[:, cc * KCH * P : (cc + 1) * KCH * P],
                        )

                quantize(w_in, emit_half, f"w{nt}")

            def gemm_tile(mt, xqT):
                out_sb = outp.tile([P, N], BF16, tag="out_sb")
                for p in range(NF):
                    psum = psump.tile(
                        [P, n_free], F32, tag="psum", name=f"ps_{mt}_{p}"
                    )
                    for j in range(KS):
                        nc.tensor.matmul(
                            psum[:],
                            xqT[j // KCH][:, j % KCH, :],
                            wqT[p][:, j, :],
                            start=(j == 0),
                            stop=(j == KS - 1),
                        )
                    sl = slice(p * n_free, (p + 1) * n_free)
                    nc.scalar.copy(out_sb[:, sl], psum[:])
                    nc.vector.tensor_tensor(
                        out_sb[:, sl], out_sb[:, sl], bias_sb[:, sl], op=OP.add
                    )
                nc.sync.dma_start(out[mt * P : (mt + 1) * P, :], out_sb[:])

            # prime the x pipeline, run the W stage (slice-major), then
            # stream all x row-tiles
            # x0 then the first wqT slice first: the head's first matmuls
            # need only these; remaining primes and W tiles follow
            xqT_early = [x_quant_transpose(0)]
            for nt in range(NPT):
                quant_w_tile(nt)
            xqT_early += [x_quant_transpose(i) for i in range(1, min(4, MT))]
            for nt in range(NPT, NT):
                quant_w_tile(nt)

            # head: run the three primed tiles slice-interleaved so each
            # wqT slice arrival unlocks 3x the PE work; per-slice output
            # staging keeps SBUF flat
            HEAD = min(4, MT)
            for p in range(NF):
                sl = slice(p * n_free, (p + 1) * n_free)
                for hm in range(HEAD):
                    psum = psump.tile(
                        [P, n_free], F32, tag="psum", name=f"ps_h{hm}_{p}"
                    )
                    xqT = xqT_early[hm]
                    for j in range(KS):
                        nc.tensor.matmul(
                            psum[:],
                            xqT[j // KCH][:, j % KCH, :],
                            wqT[p][:, j, :],
                            start=(j == 0),
                            stop=(j == KS - 1),
                        )
                    osl = outp.tile(
                        [P, n_free], BF16, tag="oslice", name=f"osl_{hm}_{p}"
                    )
                    nc.scalar.copy(osl[:], psum[:])
                    nc.vector.tensor_tensor(
                        osl[:], osl[:], bias_sb[:, sl], op=OP.add
                    )
                    nc.sync.dma_start(out[hm * P : (hm + 1) * P, sl], osl[:])
            start_mt = HEAD

            pending = {}
            for mt in range(start_mt, min(start_mt + 2, MT)):
                pending[mt] = x_quant_transpose(mt)
            for mt in range(start_mt, MT):
                ahead = mt + 2
                if start_mt + 2 <= ahead < MT:
                    pending[ahead] = x_quant_transpose(ahead)
                gemm_tile(mt, pending.pop(mt))
    return nc


# ---------------------------------------------------------------- host entry

import numpy as np
import ml_dtypes
from concourse.bass_utils import run_bass_kernel_spmd

B, S, K_IN, N_OUT = 8, 2048, 4096, 16384
M_FULL = B * S
N_CORES = 8
N_SHARD = N_OUT // N_CORES

_nc_cache = None


def _get_nc():
    global _nc_cache
    if _nc_cache is None:
        install_wait_split_hook()
        _nc_cache = build_kernel(M_FULL, K_IN, N_SHARD)
    return _nc_cache


def kernel(x, weight, bias):
    """x (8, 2048, 4096) f32; weight (16384, 4096) f32; bias (16384,) f32
    -> (8, 2048, 16384) bf16"""
    BF = ml_dtypes.bfloat16
    x = np.asarray(x)
    weight = np.asarray(weight)
    bias = np.asarray(bias)

    xb = np.ascontiguousarray(x.reshape(M_FULL, K_IN)).astype(BF)
    wb = weight.astype(BF)
    bb = bias.astype(BF)

    nc = _get_nc()
    in_maps = [
        {
            "x": xb,
            "w": np.ascontiguousarray(wb[i * N_SHARD : (i + 1) * N_SHARD]),
            "bias": np.ascontiguousarray(
                bb[i * N_SHARD : (i + 1) * N_SHARD]
            ).reshape(1, N_SHARD),
        }
        for i in range(N_CORES)
    ]
    res = run_bass_kernel_spmd(nc, in_maps, core_ids=list(range(N_CORES)))
    outs = [res.results[i]["out"] for i in range(N_CORES)]
    full = np.concatenate(outs, axis=1)  # (M_FULL, N_OUT) bf16
    return full.reshape(B, S, N_OUT)



# revision 2
# speedup vs baseline: 1.0320x; 1.0320x over previous
"""Trainium2 Bass kernel for nn_AXSLinearMixedPrecision (auto-assembled).

out = fake_quant(bf16(x)) @ fake_quant(bf16(W)).T + bf16(bias), blockwise
(block=32) absmax fake-quant on the [-15, 15] grid, bf16 GEMM + output.

Distribution over 8 NeuronCores: tensor-parallel on W rows (out_features
sharded 8 x 2048), x replicated; each core produces a 2048-wide output
column slice; the host concatenates.
"""



import orjson

from concourse import tile
from concourse.vector_clock import ScopedClock

MAX_WAITS = 1
DRAIN_MAX_WAITS = 1


def split_bir_waits(bir: dict, max_waits: int = 1) -> int:
    """Enforce <= max_waits semaphore waits per BIR instruction.

    Excess waits move onto NoOp carriers inserted immediately before the
    instruction on the same engine.  Tile emits per-proc streams in global
    tick order, so every wait references strictly-earlier work — blocking
    the dispatching engine a bit earlier cannot deadlock.
    """
    n_split = 0
    n_carrier = 0
    for f in bir.get("functions", []):
        for bb in f.get("blocks", []):
            insts = bb.get("instructions", [])
            out = []
            for ins in insts:
                si = ins.get("sync_info")
                waits = (si or {}).get("on_wait") or []
                if len(waits) > max_waits:
                    n_split += 1
                    keep = waits[len(waits) - max_waits :]
                    moved = waits[: len(waits) - max_waits]
                    for i in range(0, len(moved), max_waits):
                        n_carrier += 1
                        out.append(
                            {
                                "engine": ins.get("engine", "SP"),
                                "ins": [],
                                "outs": [],
                                "name": f"WSPLIT-{n_carrier}",
                                "opcode": "NoOp",
                                "sync_info": {
                                    "on_update": [],
                                    "on_wait": moved[i : i + max_waits],
                                },
                                "text_hint": "wait_split",
                            }
                        )
                    si["on_wait"] = keep
                out.append(ins)
            bb["instructions"] = out
    return n_split


_patched = False


def install_wait_split_hook():
    """Rewrite the BIR between bass serialization and walrus codegen."""
    global _patched
    if _patched:
        return
    _patched = True
    import concourse.bass2jax as b2j

    orig = b2j.compile_bir_kernel

    def compile_with_split(ant_bir, *args, **kwargs):
        bir = orjson.loads(ant_bir)
        split_bir_waits(bir, MAX_WAITS)
        return orig(orjson.dumps(bir), *args, **kwargs)

    b2j.compile_bir_kernel = compile_with_split


class CompatTileContext(tile.TileContext):
    def _drain_and_barrier(self, tick_clock, wait_clock):
        nc = self.nc
        drain_inst = nc.sync.drain()
        wait_clock.add_sem_waits(
            drain_inst.ins, ScopedClock({None: tick_clock.global_clock})
        )
        waits = list(drain_inst.ins.sync_info.on_wait)
        if len(waits) > DRAIN_MAX_WAITS:
            drain_inst.ins.sync_info.on_wait = waits[:DRAIN_MAX_WAITS]
            rest = waits[DRAIN_MAX_WAITS:]
            import bass_rust

            for i in range(0, len(rest), MAX_WAITS):
                nop = nc.sync.nop(nofuse=True, hint="drain_wait_split")
                nop.ins.sync_info = bass_rust.SyncInfo(
                    on_wait=rest[i : i + MAX_WAITS], on_update=[]
                )

        nc.all_engine_barrier()
        assert self.sems is not None
        popped = nc._tile_sem_poison_stack.pop()
        assert popped is self._sem_poison
        nc.clear_and_free_semaphores(list(self.sems.allocated().values()))
        nc.all_engine_barrier()



import concourse.bass as bass
import concourse.mybir as mybir

F32 = mybir.dt.float32
BF16 = mybir.dt.bfloat16
BLK = 32
QMAX = 15.0
MAGIC = 12582912.0  # 1.5 * 2**23: RNE integer rounding for |v| < 2**22
K1 = 0.0666656494140625  # floor((1/15)*2^19)/2^19 — 16-bit mantissa
K2 = 1.0172370821237564e-06  # floor(((1/15)-K1)*2^35)/2^35 — 16-bit mantissa
AX = mybir.AxisListType.X
OP = mybir.AluOpType


def build_kernel(
    M, K, N, n_free=512, dram_bufs=4, k_chunk_subtiles=8, sbuf_transpose=True
):
    P = 128
    KS = K // P         # k-subtiles
    MT = M // P         # x row tiles
    NT = N // P         # w row tiles
    NF = N // n_free    # psum tiles / wqT slices
    NB = K // BLK       # quant blocks per row
    NPT = NT // NF      # w row tiles per wqT slice
    KCH = min(k_chunk_subtiles, KS)   # k-subtiles per transpose chunk
    NKC = KS // KCH                   # transpose chunks along K
    if NKC < 2:                       # one chunk per K-half minimum
        KCH = KS // 2
        NKC = 2
    assert K % P == 0 and M % P == 0 and N % P == 0 and N % n_free == 0
    assert NT % NF == 0 and KS % KCH == 0

    nc = bass.Bass(target_bir_lowering=False)
    x = nc.dram_tensor("x", [M, K], BF16, kind="ExternalInput")
    w = nc.dram_tensor("w", [N, K], BF16, kind="ExternalInput")
    b = nc.dram_tensor("bias", [1, N], BF16, kind="ExternalInput")
    out = nc.dram_tensor("out", [M, N], BF16, kind="ExternalOutput")

    with CompatTileContext(nc) as tc:
        with (
            tc.tile_pool(name="resident", bufs=1) as resident,
            tc.tile_pool(name="xio", bufs=2) as xio,
            tc.tile_pool(name="stats", bufs=1) as stats,
            tc.tile_pool(name="t32p", bufs=1) as t32p,
            tc.tile_pool(name="xqt", bufs=4) as xqtp,
            tc.tile_pool(name="outp", bufs=2) as outp,
            tc.tile_pool(name="psum", bufs=8, space="PSUM") as psump,
            tc.tile_pool(name="dram", bufs=dram_bufs, space="DRAM") as dramp,
        ):
            wqT = [
                resident.tile([P, KS, n_free], BF16, name=f"wqT_{p}")
                for p in range(NF)
            ]
            bias_sb = resident.tile([P, N], BF16)
            nc.sync.dma_start(bias_sb[:], b[:].to_broadcast((P, N)))
            magic_sb = resident.tile([P, 1], F32)
            nc.vector.memset(magic_sb[:], MAGIC)

            def quantize(t_in, emit_half, tagn):
                """Quantize [128, K] bf16 t_in; emit_half(h, xq_half) gets
                each quantized K-half ([128, K/2] bf16) so its transposes
                start while the other half is still quantizing."""
                v_in = t_in.rearrange("p (b i) -> p b i", i=BLK)
                a = stats.tile([P, NB], F32, tag="stat_a")
                nc.vector.tensor_reduce(
                    a[:], v_in, axis=AX, op=OP.max, apply_absolute_value=True
                )
                # s = RN(amax/15) exactly: amax has an 8-bit mantissa (|bf16|),
                # so amax*K1 and amax*K2 are exact and their sum rounds to the
                # true quotient (K1+K2 carry 32 bits of 1/15).
                nc.vector.tensor_scalar_max(a[:], a[:], 1e-30)
                u1 = stats.tile([P, NB], F32, tag="stat_ur", name=f"u1_{tagn}")
                nc.vector.tensor_scalar_mul(u1[:], a[:], K1)
                s = stats.tile([P, NB], F32, tag="stat_s")
                nc.vector.scalar_tensor_tensor(
                    s[:], a[:], K2, u1[:], op0=OP.mult, op1=OP.add
                )
                r = stats.tile([P, NB], F32, tag="stat_ur", name=f"r_{tagn}")
                nc.vector.reciprocal(r[:], s[:])
                KH = K // 2
                NBH = NB // 2
                for h in range(2):
                    vh = t_in[:, h * KH : (h + 1) * KH].rearrange(
                        "p (b i) -> p b i", i=BLK
                    )
                    rh = r[:, h * NBH : (h + 1) * NBH, None].to_broadcast(
                        (P, NBH, BLK)
                    )
                    sh = s[:, h * NBH : (h + 1) * NBH, None].to_broadcast(
                        (P, NBH, BLK)
                    )
                    t = t32p.tile([P, KH], F32, tag="t32", name=f"t32_{tagn}_{h}")
                    tv = t.rearrange("p (b i) -> p b i", i=BLK)
                    nc.vector.tensor_tensor(tv, vh, rh, op=OP.mult)
                    nc.scalar.activation(
                        t[:], t[:], mybir.ActivationFunctionType.Identity,
                        bias=magic_sb[:],
                    )
                    xq_h = xio.tile(
                        [P, KH], BF16, tag="xq_sb", name=f"xqh_{tagn}_{h}"
                    )
                    qv = xq_h.rearrange("p (b i) -> p b i", i=BLK)
                    nc.vector.scalar_tensor_tensor(
                        qv, tv, MAGIC, sh, op0=OP.subtract, op1=OP.mult
                    )
                    emit_half(h, xq_h)

            def x_quant_transpose(mt):
                x_in = xio.tile([P, K], BF16, tag="x_in", name=f"x_in_{mt}")
                nc.sync.dma_start(x_in[:], x[mt * P : (mt + 1) * P, :])
                xqT = [None] * NKC
                CPH = NKC // 2  # transpose chunks per K-half

                def emit_half(h, xq_h):
                    for cc in range(CPH):
                        c = h * CPH + cc
                        xt = xqtp.tile(
                            [P, KCH, P], BF16, tag=f"xqT{c}", name=f"xqT{c}_{mt}"
                        )
                        nc.sync.dma_start_transpose(
                            xt[:], xq_h[:, cc * KCH * P : (cc + 1) * KCH * P]
                        )
                        xqT[c] = xt

                quantize(x_in, emit_half, f"x{mt}")
                return xqT

            def quant_w_tile(nt):
                p, ntl = nt // NPT, nt % NPT
                w_in = xio.tile([P, K], BF16, tag="x_in", name=f"w_in_{nt}")
                nc.sync.dma_start(w_in[:], w[nt * P : (nt + 1) * P, :])
                CPH = NKC // 2

                def emit_half(h, wq_h):
                    for cc in range(CPH):
                        c = h * CPH + cc
                        nc.sync.dma_start_transpose(
                            wqT[p][
                                :, c * KCH : (c + 1) * KCH,
                                ntl * P : (ntl + 1) * P,
                            ],
                            wq_h[:, cc * KCH * P : (cc + 1) * KCH * P],
                        )

                quantize(w_in, emit_half, f"w{nt}")

            def gemm_tile(mt, xqT):
                out_sb = outp.tile([P, N], BF16, tag="out_sb")
                for p in range(NF):
                    psum = psump.tile(
                        [P, n_free], F32, tag="psum", name=f"ps_{mt}_{p}"
                    )
                    for j in range(KS):
                        nc.tensor.matmul(
                            psum[:],
                            xqT[j // KCH][:, j % KCH, :],
                            wqT[p][:, j, :],
                            start=(j == 0),
                            stop=(j == KS - 1),
                        )
                    sl = slice(p * n_free, (p + 1) * n_free)
                    nc.scalar.copy(out_sb[:, sl], psum[:])
                    nc.vector.tensor_tensor(
                        out_sb[:, sl], out_sb[:, sl], bias_sb[:, sl], op=OP.add
                    )
                nc.sync.dma_start(out[mt * P : (mt + 1) * P, :], out_sb[:])

            # prime the x pipeline, run the W stage (slice-major), then
            # stream all x row-tiles
            # x0 then the first wqT slice first: the head's first matmuls
            # need only these; remaining primes and W tiles follow
            xqT_early = [x_quant_transpose(0)]
            for nt in range(NPT):
                quant_w_tile(nt)
            xqT_early += [x_quant_transpose(i) for i in range(1, min(4, MT))]
            for nt in range(NPT, NT):
                quant_w_tile(nt)

            # head: run the three primed tiles slice-interleaved so each
            # wqT slice arrival unlocks 3x the PE work; per-slice output
            # staging keeps SBUF flat
            HEAD = min(4, MT)
            for p in range(NF):
                sl = slice(p * n_free, (p + 1) * n_free)
                for hm in range(HEAD):
                    psum = psump.tile(
                        [P, n_free], F32, tag="psum", name=f"ps_h{hm}_{p}"
                    )
                    xqT = xqT_early[hm]
                    for j in range(KS):
                        nc.tensor.matmul(
                            psum[:],
                            xqT[j // KCH][:, j % KCH, :],
                            wqT[p][:, j, :],
                            start=(j == 0),
                            stop=(j == KS - 1),
                        )
                    osl = outp.tile(
                        [P, n_free], BF16, tag="oslice", name=f"osl_{hm}_{p}"
                    )
                    nc.scalar.copy(osl[:], psum[:])
                    nc.vector.tensor_tensor(
                        osl[:], osl[:], bias_sb[:, sl], op=OP.add
                    )
                    nc.sync.dma_start(out[hm * P : (hm + 1) * P, sl], osl[:])
            start_mt = HEAD

            pending = {}
            for mt in range(start_mt, min(start_mt + 2, MT)):
                pending[mt] = x_quant_transpose(mt)
            for mt in range(start_mt, MT):
                ahead = mt + 2
                if start_mt + 2 <= ahead < MT:
                    pending[ahead] = x_quant_transpose(ahead)
                gemm_tile(mt, pending.pop(mt))
    return nc


# ---------------------------------------------------------------- host entry

import numpy as np
import ml_dtypes
from concourse.bass_utils import run_bass_kernel_spmd

B, S, K_IN, N_OUT = 8, 2048, 4096, 16384
M_FULL = B * S
N_CORES = 8
N_SHARD = N_OUT // N_CORES

_nc_cache = None


def _get_nc():
    global _nc_cache
    if _nc_cache is None:
        install_wait_split_hook()
        _nc_cache = build_kernel(M_FULL, K_IN, N_SHARD)
    return _nc_cache


def make_in_maps(x, weight, bias):
    BF = ml_dtypes.bfloat16
    x = np.asarray(x)
    weight = np.asarray(weight)
    bias = np.asarray(bias)

    xb = np.ascontiguousarray(x.reshape(M_FULL, K_IN)).astype(BF)
    wb = weight.astype(BF)
    bb = bias.astype(BF)

    return [
        {
            "x": xb,
            "w": np.ascontiguousarray(wb[i * N_SHARD : (i + 1) * N_SHARD]),
            "bias": np.ascontiguousarray(
                bb[i * N_SHARD : (i + 1) * N_SHARD]
            ).reshape(1, N_SHARD),
        }
        for i in range(N_CORES)
    ]


def kernel(x, weight, bias):
    """x (8, 2048, 4096) f32; weight (16384, 4096) f32; bias (16384,) f32
    -> (8, 2048, 16384) bf16"""
    nc = _get_nc()
    in_maps = make_in_maps(x, weight, bias)
    res = run_bass_kernel_spmd(nc, in_maps, core_ids=list(range(N_CORES)))
    outs = [res.results[i]["out"] for i in range(N_CORES)]
    full = np.concatenate(outs, axis=1)  # (M_FULL, N_OUT) bf16
    return full.reshape(B, S, N_OUT)

